# revision 1
# baseline (speedup 1.0000x reference)
"""Trainium2 Bass kernel for nn_CIN (Compressed Interaction Network).

Math (per layer k, x0 = x fixed):
    x_{k+1}[b,h,d] = sum_{i,j} W[i,j,h] * x0[b,i,d] * xk[b,j,d]
    outs_k[b,h]    = sum_d x_{k+1}[b,h,d]
    output = concat(outs_0, outs_1, outs_2)   # [B, 384]

Strategy (pure data parallel over batch, 8 cores x 128 batches):
  - All compute in bf16 with fp32 PSUM accumulation.
  - Per core, 16 "blocks" of 8 batches; free dim F = 8*64 = 512 (b,d).
  - Product tensor P[(i,j), f] = x0[i,f]*xk[j,f] built chunk-by-chunk on the
    vector engine: chunk i is xk_tile * REP_i, where REP_i = x0 row i
    broadcast across all 128 partitions.
  - REP_i tiles made by DMA (stride-0 partition reads from DRAM) and gpsimd
    partition_broadcast (from a partition-0 staging tile), split to balance.
  - Layer 0 reuses the same structure with j' in [0,128): xk0[j'] = x[j' mod 40]
    and host-prescaled W0 (1/4 or 1/3 per column class) so the triple-counting
    cancels exactly.
  - Matmuls: stationary = W chunk [j(=c),h], moving = P chunk [c, 512],
    40-chunk PSUM accumulation -> x_{k+1} in [h, (b,d)] layout, which is
    exactly the xk layout the next layer needs.
  - Layer 2 never materializes x3: outs_2 = W2 : G2 where
    G2'[b][j,i] = sum_d x2[b,j,d]*x0[b,i,d] (small per-batch Gram via PE),
    then one 40-chunk contraction. Saves 43% of FLOPs and 1/3 of the
    elementwise work.
"""
import os
import sys

sys.path.insert(0, "/opt/trn_rl_repo")
os.environ.setdefault("JAX_PLATFORMS", "cpu")

from contextlib import ExitStack

import numpy as np
import ml_dtypes

import concourse.bass as bass  # noqa: F401  (import keeps bass registered)
import concourse.tile as tile
from concourse import bacc, library_config, mybir
from concourse.bass_utils import run_bass_kernel_spmd

BF16 = mybir.dt.bfloat16
F32 = mybir.dt.float32
NPBF16 = ml_dtypes.bfloat16

B, M, D, HK = 1024, 40, 64, 128
NCORE = 8
BS = B // NCORE          # 128 batches per core
NBLK = 16                # blocks per core
BB = BS // NBLK          # 8 batches per block
F = BB * D               # 512 free elements per block
NI = M                   # 40 chunks per layer
GRP = 4                  # REP tiles per group (one DMA per group)
NGRP = NI // GRP         # 10 groups
GPS_GROUPS = 4           # trailing groups generated by gpsimd partition_broadcast
GPS_I0 = (NGRP - GPS_GROUPS) * GRP  # first gpsimd-generated i
# number of leading TT chunks per layer routed to gpsimd instead of DVE
GPS_TT = 0

_PROFILE = False
_TRACE_KW = {}
_nc_cache = None
_last_results = None


def _build():
    nc = bacc.Bacc("TRN2", target_bir_lowering=False, debug=False,
                   enable_asserts=False)

    xt_d = nc.dram_tensor("xt", [128, NBLK, F], BF16, kind="ExternalInput").ap()
    x0f_d = nc.dram_tensor("x0f", [NBLK, NI, F], BF16, kind="ExternalInput").ap()
    xdt_d = nc.dram_tensor("xdt", [D, BS, M], BF16, kind="ExternalInput").ap()
    w0_d = nc.dram_tensor("w0", [128, NI, HK], BF16, kind="ExternalInput").ap()
    w1_d = nc.dram_tensor("w1", [128, NI, HK], BF16, kind="ExternalInput").ap()
    w2_d = nc.dram_tensor("w2", [128, NI, HK], BF16, kind="ExternalInput").ap()
    idb_d = nc.dram_tensor("idb", [128, 128], BF16, kind="ExternalInput").ap()
    idf_d = nc.dram_tensor("idf", [128, 128], F32, kind="ExternalInput").ap()
    out_d = nc.dram_tensor("out", [BS, 3 * HK], F32, kind="ExternalOutput").ap()

    with tile.TileContext(nc) as tc, ExitStack() as ctx:
        stat = ctx.enter_context(tc.tile_pool(name="stat", bufs=1))
        xtp = ctx.enter_context(tc.tile_pool(name="xtp", bufs=3))
        x0sp = ctx.enter_context(tc.tile_pool(name="x0sp", bufs=2))
        repp = ctx.enter_context(tc.tile_pool(name="repp", bufs=12))
        pp = ctx.enter_context(tc.tile_pool(name="pp", bufs=8))
        xkp = ctx.enter_context(tc.tile_pool(name="xkp", bufs=4))
        x2tp = ctx.enter_context(tc.tile_pool(name="x2tp", bufs=3))
        ps_acc = ctx.enter_context(tc.tile_pool(name="ps_acc", bufs=2, space="PSUM"))
        ps_tr = ctx.enter_context(tc.tile_pool(name="ps_tr", bufs=2, space="PSUM"))
        ps_g2 = ctx.enter_context(tc.tile_pool(name="ps_g2", bufs=2, space="PSUM"))
        ps_sm = ctx.enter_context(tc.tile_pool(name="ps_sm", bufs=2, space="PSUM"))

        if GPS_GROUPS > 0 or GPS_TT > 0:
            nc.gpsimd.load_library(library_config.attn)

        #静 static tensors
        w0sb = stat.tile([128, NI, HK], BF16, tag="w0sb")
        nc.sync.dma_start(w0sb[:], w0_d[:])
        w1sb = stat.tile([128, NI, HK], BF16, tag="w1sb")
        nc.sync.dma_start(w1sb[:], w1_d[:])
        w2sb = stat.tile([128, NI, HK], BF16, tag="w2sb")
        nc.sync.dma_start(w2sb[:], w2_d[:])
        xdt_sb = stat.tile([D, BS, M], BF16, tag="xdt_sb")
        nc.sync.dma_start(xdt_sb[:], xdt_d[:])
        idb = stat.tile([128, 128], BF16, tag="idb")
        nc.sync.dma_start(idb[:], idb_d[:])
        idf = stat.tile([128, 128], F32, tag="idf")
        nc.sync.dma_start(idf[:], idf_d[:])
        g2stack = stat.tile([128, NI, BS], BF16, tag="g2stack")
        outs_sb = stat.tile([128, 3, BS], F32, tag="outs_sb")
        outT_sb = stat.tile([128, 3, HK], F32, tag="outT_sb")

        for blk in range(NBLK):
            xt_t = xtp.tile([128, F], BF16, tag="xt")
            nc.sync.dma_start(xt_t[:], xt_d[:, blk, :])

            # staging rows for gpsimd broadcasts (partition 0 only)
            if GPS_GROUPS > 0:
                x0s_t = x0sp.tile([1, NI - GPS_I0, F], BF16, tag="x0s")
                nc.sync.dma_start(x0s_t[:], x0f_d[blk:blk + 1, GPS_I0:NI, :])

            # REP tiles: x0 rows broadcast across partitions
            rep_grps = []
            for g in range(NGRP):
                rg = repp.tile([128, GRP, F], BF16, tag="rep")
                if g >= NGRP - GPS_GROUPS:
                    for e in range(GRP):
                        i = g * GRP + e
                        nc.gpsimd.partition_broadcast(
                            rg[:, e, :], x0s_t[0:1, i - GPS_I0, :])
                else:
                    eng = nc.sync if g % 2 == 0 else nc.scalar
                    eng.dma_start(
                        rg[:], x0f_d[blk:blk + 1, g * GRP:(g + 1) * GRP, :]
                        .partition_broadcast(128))
                rep_grps.append(rg)

            def rep_ap(i):
                g, e = divmod(i, GRP)
                return rep_grps[g][:, e, :]

            # ---- layer 0 (tiled j' in [0,128), host-prescaled W0) ----
            x1ps = ps_acc.tile([128, F], F32, tag="acc")
            for i in range(NI):
                p_t = pp.tile([128, F], BF16, tag="p")
                eng = nc.gpsimd if i < GPS_TT else nc.vector
                eng.tensor_mul(p_t[:], xt_t[:], rep_ap(i))
                nc.tensor.matmul(x1ps[:], w0sb[:, i, :], p_t[:],
                                 start=(i == 0), stop=(i == NI - 1))
            x1sb = xkp.tile([128, F], BF16, tag="xk")
            nc.scalar.copy(x1sb[:], x1ps[:])
            nc.vector.tensor_reduce(
                outs_sb[:, 0, blk * BB:(blk + 1) * BB],
                x1sb[:].rearrange("p (b d) -> p b d", d=D),
                axis=mybir.AxisListType.X, op=mybir.AluOpType.add)

            # ---- layer 1 ----
            x2ps = ps_acc.tile([128, F], F32, tag="acc")
            for i in range(NI):
                p_t = pp.tile([128, F], BF16, tag="p")
                eng = nc.gpsimd if i < GPS_TT else nc.vector
                eng.tensor_mul(p_t[:], x1sb[:], rep_ap(i))
                nc.tensor.matmul(x2ps[:], w1sb[:, i, :], p_t[:],
                                 start=(i == 0), stop=(i == NI - 1))
            x2sb = xkp.tile([128, F], BF16, tag="xk")
            nc.scalar.copy(x2sb[:], x2ps[:])
            nc.vector.tensor_reduce(
                outs_sb[:, 1, blk * BB:(blk + 1) * BB],
                x2sb[:].rearrange("p (b d) -> p b d", d=D),
                axis=mybir.AxisListType.X, op=mybir.AluOpType.add)

            # ---- layer 2: per-batch Gram G2'[b][j,i] = sum_d x2[j,d]*x0[i,d] ----
            for b8 in range(BB):
                b = blk * BB + b8
                x2t_ps = ps_tr.tile([D, 128], BF16, tag="x2t")
                nc.tensor.transpose(x2t_ps[:], x2sb[:, b8 * D:(b8 + 1) * D], idb[:])
                x2t = x2tp.tile([D, 128], BF16, tag="x2t_sb")
                nc.scalar.copy(x2t[:], x2t_ps[:])
                g2ps = ps_g2.tile([128, NI], F32, tag="g2")
                nc.tensor.matmul(g2ps[:], x2t[:], xdt_sb[:, b, :],
                                 start=True, stop=True)
                nc.scalar.copy(g2stack[:, :, b], g2ps[:])

        # ---- outs_2 = W2 : G2 ----
        out2ps = ps_sm.tile([HK, BS], F32, tag="sm")
        for i in range(NI):
            nc.tensor.matmul(out2ps[:], w2sb[:, i, :], g2stack[:, i, :],
                             start=(i == 0), stop=(i == NI - 1))
        nc.scalar.copy(outs_sb[:, 2, :], out2ps[:])

        # ---- transpose [h, b] -> [b, h] and store ----
        for k in range(3):
            trp = ps_sm.tile([128, 128], F32, tag="sm")
            nc.tensor.transpose(trp[:], outs_sb[:, k, :], idf[:])
            nc.scalar.copy(outT_sb[:, k, :], trp[:])
        nc.sync.dma_start(out_d[:], outT_sb[:])

    nc.compile()
    return nc


def _host_prep(x, W0, W1, W2):
    """Build per-core input maps. All reshapes/casts in numpy."""
    jmod = np.arange(128) % M
    wgt = np.where(jmod < 128 - 3 * M, 0.25, 1.0 / 3.0).astype(np.float32)
    # W0': [j'=128, i, h], prescaled so the j'-tiling triple count cancels
    w0p = (W0[:, jmod, :] * wgt[None, :, None]).transpose(1, 0, 2)
    w0p = np.ascontiguousarray(w0p).astype(NPBF16)
    w1t = np.ascontiguousarray(W1.transpose(1, 0, 2)).astype(NPBF16)
    w2t = np.ascontiguousarray(W2.transpose(1, 0, 2)).astype(NPBF16)
    idb = np.eye(128, dtype=np.float32).astype(NPBF16)
    idf = np.eye(128, dtype=np.float32)

    xbf = x.astype(NPBF16)
    in_maps = []
    for c in range(NCORE):
        xs = xbf[c * BS:(c + 1) * BS]                     # [BS, M, D]
        xsT = xs.transpose(1, 0, 2)                       # [M, BS, D]
        xt_full = xsT[jmod]                               # [128, BS, D]
        xt = np.ascontiguousarray(xt_full).reshape(128, NBLK, F)
        x0f = np.ascontiguousarray(
            xsT.reshape(M, NBLK, F).transpose(1, 0, 2))   # [NBLK, M, F]
        xdt = np.ascontiguousarray(xs.transpose(2, 0, 1))  # [D, BS, M]
        in_maps.append({
            "xt": xt, "x0f": x0f, "xdt": xdt,
            "w0": w0p, "w1": w1t, "w2": w2t,
            "idb": idb, "idf": idf,
        })
    return in_maps


def kernel(x, W0, W1, W2):
    global _nc_cache, _last_results
    x = np.asarray(x, dtype=np.float32)
    W0 = np.asarray(W0, dtype=np.float32)
    W1 = np.asarray(W1, dtype=np.float32)
    W2 = np.asarray(W2, dtype=np.float32)

    if _nc_cache is None:
        _nc_cache = _build()
    nc = _nc_cache

    in_maps = _host_prep(x, W0, W1, W2)
    res = run_bass_kernel_spmd(nc, in_maps, list(range(NCORE)),
                               trace=_PROFILE, **_TRACE_KW)
    _last_results = res
    out = np.concatenate(
        [np.asarray(res.results[c]["out"]) for c in range(NCORE)], axis=0)
    return out.astype(np.float32)


# revision 2
# speedup vs baseline: 1.0176x; 1.0176x over previous
"""Trainium2 Bass kernel for nn_CIN (Compressed Interaction Network).

Math (per layer k, x0 = x fixed):
    x_{k+1}[b,h,d] = sum_{i,j} W[i,j,h] * x0[b,i,d] * xk[b,j,d]
    outs_k[b,h]    = sum_d x_{k+1}[b,h,d]
    output = concat(outs_0, outs_1, outs_2)   # [B, 384]

Strategy (pure data parallel over batch, 8 cores x 128 batches):
  - bf16 compute, fp32 PSUM accumulation.
  - Per core, 8 blocks of 16 batches; free dim F = 16*64 = 1024 (b,d).
  - Product tensor P[(i,j), f] = x0[i,f]*xk[j,f] built as fused "group"
    tensor_tensor ops: one instruction covers 4 i-chunks via a stride-0
    broadcast middle dim on the xk operand (keeps DVE 2x bf16 mode, amortizes
    per-op overhead and semaphores 4x). Split between VectorE and GpSimd.
  - REP_i tiles (x0 row i broadcast across 128 partitions) via DMA with
    stride-0 first-dim APs reading DRAM, grouped 4 tiles per DMA, alternating
    the two HWDGE rings (sync / scalar).
  - Layer 0 reuses the same structure with j' in [0,128): xk0[j'] = x[j' mod 40]
    and host-prescaled W0 (1/4 or 1/3 per column class) so the triple count
    cancels exactly.
  - Matmuls: stationary = W chunk [j(=c),h], moving = P chunk [c, 512],
    40-chunk PSUM accumulation -> x_{k+1} in [h, (b,d)] layout, which is
    exactly the xk layout the next layer needs.
  - Layer 2 never materializes x3: outs_2 = W2 : G2 where
    G2'[b][j,i] = sum_d x2[b,j,d]*x0[b,i,d] (small per-batch Gram via PE),
    then one 40-chunk contraction. Saves 43% of FLOPs and 1/3 of the
    elementwise work.
"""
import os
import sys

sys.path.insert(0, "/opt/trn_rl_repo")
os.environ.setdefault("JAX_PLATFORMS", "cpu")

from contextlib import ExitStack

import numpy as np
import ml_dtypes

import concourse.bass as bass  # noqa: F401
import concourse.tile as tile
from concourse import bacc, mybir
from concourse.bass_utils import run_bass_kernel_spmd

BF16 = mybir.dt.bfloat16
F32 = mybir.dt.float32
NPBF16 = ml_dtypes.bfloat16

B, M, D, HK = 1024, 40, 64, 128
NCORE = 8
BS = B // NCORE          # 128 batches per core
NBLK = 8                 # blocks per core
BB = BS // NBLK          # 16 batches per block
F = BB * D               # 1024 free elements per block
NI = M                   # 40 chunks per layer
GRP = 4                  # chunks fused per group (one TT / one REP DMA)
NGRP = NI // GRP         # 10 groups
GPS_TT = 2               # groups per layer routed to gpsimd
NMM = F // 512           # matmuls per chunk (PSUM bank = 512 fp32)

_PROFILE = False
_TRACE_KW = {}
_nc_cache = None
_last_results = None


def _build():
    nc = bacc.Bacc("TRN2", target_bir_lowering=False, debug=False,
                   enable_asserts=False)

    xt_d = nc.dram_tensor("xt", [128, NBLK, F], BF16, kind="ExternalInput").ap()
    x0f_d = nc.dram_tensor("x0f", [NBLK, NI, F], BF16, kind="ExternalInput").ap()
    xdt_d = nc.dram_tensor("xdt", [D, BS, M], BF16, kind="ExternalInput").ap()
    w0_d = nc.dram_tensor("w0", [128, NI, HK], BF16, kind="ExternalInput").ap()
    w1_d = nc.dram_tensor("w1", [128, NI, HK], BF16, kind="ExternalInput").ap()
    w2_d = nc.dram_tensor("w2", [128, NI, HK], BF16, kind="ExternalInput").ap()
    idb_d = nc.dram_tensor("idb", [128, 128], BF16, kind="ExternalInput").ap()
    idf_d = nc.dram_tensor("idf", [128, 128], F32, kind="ExternalInput").ap()
    out_d = nc.dram_tensor("out", [BS, 3 * HK], F32, kind="ExternalOutput").ap()

    with tile.TileContext(nc) as tc, ExitStack() as ctx:
        stat = ctx.enter_context(tc.tile_pool(name="stat", bufs=1))
        xtp = ctx.enter_context(tc.tile_pool(name="xtp", bufs=3))
        repp = ctx.enter_context(tc.tile_pool(name="repp", bufs=NGRP))
        pp = ctx.enter_context(tc.tile_pool(name="pp", bufs=3))
        xkp = ctx.enter_context(tc.tile_pool(name="xkp", bufs=4))
        x2tp = ctx.enter_context(tc.tile_pool(name="x2tp", bufs=3))
        ps_acc = ctx.enter_context(tc.tile_pool(name="ps_acc", bufs=2, space="PSUM"))
        ps_tr = ctx.enter_context(tc.tile_pool(name="ps_tr", bufs=2, space="PSUM"))
        ps_sm = ctx.enter_context(tc.tile_pool(name="ps_sm", bufs=2, space="PSUM"))

        w0sb = stat.tile([128, NI, HK], BF16, tag="w0sb")
        nc.sync.dma_start(w0sb[:], w0_d[:])
        w1sb = stat.tile([128, NI, HK], BF16, tag="w1sb")
        nc.sync.dma_start(w1sb[:], w1_d[:])
        w2sb = stat.tile([128, NI, HK], BF16, tag="w2sb")
        nc.sync.dma_start(w2sb[:], w2_d[:])
        xdt_sb = stat.tile([D, BS, M], BF16, tag="xdt_sb")
        nc.sync.dma_start(xdt_sb[:], xdt_d[:])
        idb = stat.tile([128, 128], BF16, tag="idb")
        nc.sync.dma_start(idb[:], idb_d[:])
        idf = stat.tile([128, 128], F32, tag="idf")
        nc.sync.dma_start(idf[:], idf_d[:])
        g2stack = stat.tile([128, NI, BS], BF16, tag="g2stack")
        outs_sb = stat.tile([128, 3, BS], F32, tag="outs_sb")
        outT_sb = stat.tile([128, 3, HK], F32, tag="outT_sb")

        for blk in range(NBLK):
            xt_t = xtp.tile([128, F], BF16, tag="xt")
            nc.sync.dma_start(xt_t[:], xt_d[:, blk, :])

            # REP groups: x0 rows broadcast across partitions (DMA stride-0)
            rep_grps = []
            for g in range(NGRP):
                rg = repp.tile([128, GRP, F], BF16, tag="rep")
                eng = nc.sync if g % 2 == 0 else nc.scalar
                eng.dma_start(
                    rg[:], x0f_d[blk:blk + 1, g * GRP:(g + 1) * GRP, :]
                    .partition_broadcast(128))
                rep_grps.append(rg)

            def layer(src_t, wsb, acc):
                """P-groups + 40-chunk accumulate matmul."""
                src_b = src_t[:].unsqueeze(1).broadcast_to([128, GRP, F])
                for g in range(NGRP):
                    p_t = pp.tile([128, GRP, F], BF16, tag="p")
                    eng = nc.gpsimd if g < GPS_TT else nc.vector
                    eng.tensor_mul(p_t[:], src_b, rep_grps[g][:])
                    for e in range(GRP):
                        i = g * GRP + e
                        for s in range(NMM):
                            nc.tensor.matmul(
                                acc[:, s * 512:(s + 1) * 512],
                                wsb[:, i, :],
                                p_t[:, e, s * 512:(s + 1) * 512],
                                start=(i == 0), stop=(i == NI - 1))

            # ---- layer 0 ----
            x1ps = ps_acc.tile([128, F], F32, tag="acc")
            layer(xt_t, w0sb, x1ps)
            x1sb = xkp.tile([128, F], BF16, tag="xk")
            nc.scalar.copy(x1sb[:], x1ps[:])
            nc.vector.tensor_reduce(
                outs_sb[:, 0, blk * BB:(blk + 1) * BB],
                x1sb[:].rearrange("p (b d) -> p b d", d=D),
                axis=mybir.AxisListType.X, op=mybir.AluOpType.add)

            # ---- layer 1 ----
            x2ps = ps_acc.tile([128, F], F32, tag="acc")
            layer(x1sb, w1sb, x2ps)
            x2sb = xkp.tile([128, F], BF16, tag="xk")
            nc.scalar.copy(x2sb[:], x2ps[:])
            nc.vector.tensor_reduce(
                outs_sb[:, 1, blk * BB:(blk + 1) * BB],
                x2sb[:].rearrange("p (b d) -> p b d", d=D),
                axis=mybir.AxisListType.X, op=mybir.AluOpType.add)

            # ---- layer 2: per-batch Gram G2'[b][j,i] = sum_d x2[j,d]*x0[i,d] ----
            for b8 in range(BB):
                b = blk * BB + b8
                x2t_ps = ps_tr.tile([D, 128], BF16, tag="x2t")
                nc.tensor.transpose(x2t_ps[:], x2sb[:, b8 * D:(b8 + 1) * D], idb[:])
                x2t = x2tp.tile([D, 128], BF16, tag="x2t_sb")
                nc.scalar.copy(x2t[:], x2t_ps[:])
                g2ps = ps_sm.tile([128, NI], F32, tag="sm")
                nc.tensor.matmul(g2ps[:], x2t[:], xdt_sb[:, b, :],
                                 start=True, stop=True)
                nc.scalar.copy(g2stack[:, :, b], g2ps[:])

        # ---- outs_2 = W2 : G2 ----
        out2ps = ps_sm.tile([HK, BS], F32, tag="sm")
        for i in range(NI):
            nc.tensor.matmul(out2ps[:], w2sb[:, i, :], g2stack[:, i, :],
                             start=(i == 0), stop=(i == NI - 1))
        nc.scalar.copy(outs_sb[:, 2, :], out2ps[:])

        # ---- transpose [h, b] -> [b, h] and store ----
        for k in range(3):
            trp = ps_sm.tile([128, 128], F32, tag="sm")
            nc.tensor.transpose(trp[:], outs_sb[:, k, :], idf[:])
            nc.scalar.copy(outT_sb[:, k, :], trp[:])
        nc.sync.dma_start(out_d[:], outT_sb[:])

    nc.compile()
    return nc


def _host_prep(x, W0, W1, W2):
    """Build per-core input maps. All reshapes/casts in numpy."""
    jmod = np.arange(128) % M
    wgt = np.where(jmod < 128 - 3 * M, 0.25, 1.0 / 3.0).astype(np.float32)
    w0p = (W0[:, jmod, :] * wgt[None, :, None]).transpose(1, 0, 2)
    w0p = np.ascontiguousarray(w0p).astype(NPBF16)
    w1t = np.ascontiguousarray(W1.transpose(1, 0, 2)).astype(NPBF16)
    w2t = np.ascontiguousarray(W2.transpose(1, 0, 2)).astype(NPBF16)
    idb = np.eye(128, dtype=np.float32).astype(NPBF16)
    idf = np.eye(128, dtype=np.float32)

    xbf = x.astype(NPBF16)
    in_maps = []
    for c in range(NCORE):
        xs = xbf[c * BS:(c + 1) * BS]                     # [BS, M, D]
        xsT = xs.transpose(1, 0, 2)                       # [M, BS, D]
        xt = np.ascontiguousarray(xsT[jmod]).reshape(128, NBLK, F)
        x0f = np.ascontiguousarray(
            xsT.reshape(M, NBLK, F).transpose(1, 0, 2))   # [NBLK, M, F]
        xdt = np.ascontiguousarray(xs.transpose(2, 0, 1))  # [D, BS, M]
        in_maps.append({
            "xt": xt, "x0f": x0f, "xdt": xdt,
            "w0": w0p, "w1": w1t, "w2": w2t,
            "idb": idb, "idf": idf,
        })
    return in_maps


def kernel(x, W0, W1, W2):
    global _nc_cache, _last_results
    x = np.asarray(x, dtype=np.float32)
    W0 = np.asarray(W0, dtype=np.float32)
    W1 = np.asarray(W1, dtype=np.float32)
    W2 = np.asarray(W2, dtype=np.float32)

    if _nc_cache is None:
        _nc_cache = _build()
    nc = _nc_cache

    in_maps = _host_prep(x, W0, W1, W2)
    res = run_bass_kernel_spmd(nc, in_maps, list(range(NCORE)),
                               trace=_PROFILE, **_TRACE_KW)
    _last_results = res
    out = np.concatenate(
        [np.asarray(res.results[c]["out"]) for c in range(NCORE)], axis=0)
    return out.astype(np.float32)


# revision 3
# speedup vs baseline: 1.6615x; 1.6328x over previous
"""Trainium2 Bass kernel for nn_CIN (Compressed Interaction Network).

Math (per layer k, x0 = x fixed):
    x_{k+1}[b,h,d] = sum_{i,j} W[i,j,h] * x0[b,i,d] * xk[b,j,d]
    outs_k[b,h]    = sum_d x_{k+1}[b,h,d]
    output = concat(outs_0, outs_1, outs_2)   # [B, 384]

Strategy (pure data parallel over batch, 8 cores x 128 batches):
  - bf16 compute, fp32 PSUM accumulation.
  - Per core, 8 blocks of 16 batches; free dim F = 16*64 = 1024 (b,d).
  - Layer 0 uses the i<=j symmetry: 820 unique pairs, W0sym = W0[i,j]+W0[j,i]
    (host-folded), pair products built from two host-prepared gather images
    (XSUF = x[j(c)], RSUF = x[i(c)]) -> only 2 fused tensor_tensor ops and
    16 matmuls per block.
  - Layer 1 products P[(i,j), f] = x0[i,f]*x1[j,f]: REP_i tiles (x0 row i
    broadcast across partitions) made by DMA with stride-0 first-dim APs,
    4 tiles per DMA, alternating the two HWDGE rings; products via fused
    group tensor_tensor (one instr = 4 chunks, stride-0 middle dim on the
    x1 operand keeps the DVE 2x bf16 mode and amortizes overhead 4x).
  - Matmuls: stationary = W chunk [c,h], moving = P chunk [c, 512], PSUM
    accumulation -> x_{k+1} in [h, (b,d)] layout = next layer's input layout.
  - Layer 2 never materializes x3: outs_2 = W2 : G2 where
    G2'[b][j,i] = sum_d x2[b,j,d]*x0[b,i,d] (small per-batch Gram via PE),
    then one 40-chunk contraction. Saves 43% of FLOPs and a third of the
    elementwise work.
"""
import os
import sys

sys.path.insert(0, "/opt/trn_rl_repo")
os.environ.setdefault("JAX_PLATFORMS", "cpu")

from contextlib import ExitStack

import numpy as np
import ml_dtypes

import concourse.bass as bass  # noqa: F401
import concourse.tile as tile
from concourse import bacc, mybir
from concourse.bass_utils import run_bass_kernel_spmd

BF16 = mybir.dt.bfloat16
F32 = mybir.dt.float32
NPBF16 = ml_dtypes.bfloat16

B, M, D, HK = 1024, 40, 64, 128
NCORE = 8
BS = B // NCORE          # 128 batches per core
NBLK = 8                 # blocks per core
BB = BS // NBLK          # 16 batches per block
F = BB * D               # 1024 free elements per block
NI = M                   # 40 chunks in layer 1
GRP = 4                  # chunks fused per group (one TT / one REP DMA)
NGRP = NI // GRP         # 10 groups in layer 1
NP0 = M * (M + 1) // 2   # 820 unique layer-0 pairs
NCH0 = 8                 # layer-0 chunks (820 -> 7 used + zero pad to 8)
NGRP0 = NCH0 // GRP      # 2 layer-0 groups
NMM = F // 512           # matmuls per chunk (PSUM bank = 512 fp32)

_PROFILE = False
_TRACE_KW = {}
_nc_cache = None
_last_results = None


def _build():
    nc = bacc.Bacc("TRN2", target_bir_lowering=False, debug=False,
                   enable_asserts=False)

    xsuf_d = nc.dram_tensor("xsuf", [NBLK, NGRP0, GRP, 128, F], BF16,
                            kind="ExternalInput").ap()
    rsuf_d = nc.dram_tensor("rsuf", [NBLK, NGRP0, GRP, 128, F], BF16,
                            kind="ExternalInput").ap()
    x0f_d = nc.dram_tensor("x0f", [NBLK, NI, F], BF16, kind="ExternalInput").ap()
    xdt_d = nc.dram_tensor("xdt", [D, BS, M], BF16, kind="ExternalInput").ap()
    w0_d = nc.dram_tensor("w0", [NCH0, 128, HK], BF16, kind="ExternalInput").ap()
    w1_d = nc.dram_tensor("w1", [128, NI, HK], BF16, kind="ExternalInput").ap()
    w2_d = nc.dram_tensor("w2", [128, NI, HK], BF16, kind="ExternalInput").ap()
    idb_d = nc.dram_tensor("idb", [128, 128], BF16, kind="ExternalInput").ap()
    idf_d = nc.dram_tensor("idf", [128, 128], F32, kind="ExternalInput").ap()
    out_d = nc.dram_tensor("out", [BS, 3 * HK], F32, kind="ExternalOutput").ap()

    with tile.TileContext(nc) as tc, ExitStack() as ctx:
        stat = ctx.enter_context(tc.tile_pool(name="stat", bufs=1))
        sufp = ctx.enter_context(tc.tile_pool(name="sufp", bufs=4))
        repp = ctx.enter_context(tc.tile_pool(name="repp", bufs=9))
        pp = ctx.enter_context(tc.tile_pool(name="pp", bufs=3))
        xkp = ctx.enter_context(tc.tile_pool(name="xkp", bufs=4))
        x2tp = ctx.enter_context(tc.tile_pool(name="x2tp", bufs=3))
        ps_acc = ctx.enter_context(tc.tile_pool(name="ps_acc", bufs=2, space="PSUM"))
        ps_tr = ctx.enter_context(tc.tile_pool(name="ps_tr", bufs=2, space="PSUM"))
        ps_sm = ctx.enter_context(tc.tile_pool(name="ps_sm", bufs=2, space="PSUM"))

        w0sb = stat.tile([128, NCH0, HK], BF16, tag="w0sb")
        nc.sync.dma_start(w0sb[:], w0_d.rearrange("c p h -> p c h"))
        w1sb = stat.tile([128, NI, HK], BF16, tag="w1sb")
        nc.sync.dma_start(w1sb[:], w1_d[:])
        w2sb = stat.tile([128, NI, HK], BF16, tag="w2sb")
        nc.sync.dma_start(w2sb[:], w2_d[:])
        xdt_sb = stat.tile([D, BS, M], BF16, tag="xdt_sb")
        nc.sync.dma_start(xdt_sb[:], xdt_d[:])
        idb = stat.tile([128, 128], BF16, tag="idb")
        nc.sync.dma_start(idb[:], idb_d[:])
        idf = stat.tile([128, 128], F32, tag="idf")
        nc.sync.dma_start(idf[:], idf_d[:])
        g2stack = stat.tile([128, NI, BS], BF16, tag="g2stack")
        outs_sb = stat.tile([128, 3, BS], F32, tag="outs_sb")
        outT_sb = stat.tile([128, 3, HK], F32, tag="outT_sb")

        for blk in range(NBLK):
            # ---- layer 0 (symmetric pairs) ----
            x1ps = ps_acc.tile([128, F], F32, tag="acc")
            for g in range(NGRP0):
                xs_t = sufp.tile([128, GRP, F], BF16, tag="suf")
                nc.sync.dma_start(
                    xs_t[:], xsuf_d[blk, g].rearrange("e p f -> p e f"))
                rs_t = sufp.tile([128, GRP, F], BF16, tag="suf")
                nc.scalar.dma_start(
                    rs_t[:], rsuf_d[blk, g].rearrange("e p f -> p e f"))
                p_t = pp.tile([128, GRP, F], BF16, tag="p")
                nc.vector.tensor_mul(p_t[:], xs_t[:], rs_t[:])
                for e in range(GRP):
                    ch = g * GRP + e
                    for s in range(NMM):
                        nc.tensor.matmul(
                            x1ps[:, s * 512:(s + 1) * 512],
                            w0sb[:, ch, :],
                            p_t[:, e, s * 512:(s + 1) * 512],
                            start=(ch == 0), stop=(ch == NCH0 - 1))
            x1sb = xkp.tile([128, F], BF16, tag="xk")
            nc.scalar.copy(x1sb[:], x1ps[:])
            nc.vector.tensor_reduce(
                outs_sb[:, 0, blk * BB:(blk + 1) * BB],
                x1sb[:].rearrange("p (b d) -> p b d", d=D),
                axis=mybir.AxisListType.X, op=mybir.AluOpType.add)

            # REP groups for layer 1: x0 rows broadcast across partitions
            rep_grps = []
            for g in range(NGRP):
                rg = repp.tile([128, GRP, F], BF16, tag="rep")
                eng = nc.sync if g % 2 == 0 else nc.scalar
                eng.dma_start(
                    rg[:], x0f_d[blk:blk + 1, g * GRP:(g + 1) * GRP, :]
                    .partition_broadcast(128))
                rep_grps.append(rg)

            # ---- layer 1 ----
            x2ps = ps_acc.tile([128, F], F32, tag="acc")
            x1b = x1sb[:].unsqueeze(1).broadcast_to([128, GRP, F])
            for g in range(NGRP):
                p_t = pp.tile([128, GRP, F], BF16, tag="p")
                nc.vector.tensor_mul(p_t[:], x1b, rep_grps[g][:])
                for e in range(GRP):
                    i = g * GRP + e
                    for s in range(NMM):
                        nc.tensor.matmul(
                            x2ps[:, s * 512:(s + 1) * 512],
                            w1sb[:, i, :],
                            p_t[:, e, s * 512:(s + 1) * 512],
                            start=(i == 0), stop=(i == NI - 1))
            x2sb = xkp.tile([128, F], BF16, tag="xk")
            nc.scalar.copy(x2sb[:], x2ps[:])
            nc.vector.tensor_reduce(
                outs_sb[:, 1, blk * BB:(blk + 1) * BB],
                x2sb[:].rearrange("p (b d) -> p b d", d=D),
                axis=mybir.AxisListType.X, op=mybir.AluOpType.add)

            # ---- layer 2: per-batch Gram G2'[b][j,i] = sum_d x2[j,d]*x0[i,d] ----
            for b8 in range(BB):
                b = blk * BB + b8
                x2t_ps = ps_tr.tile([D, 128], BF16, tag="x2t")
                nc.tensor.transpose(x2t_ps[:], x2sb[:, b8 * D:(b8 + 1) * D], idb[:])
                x2t = x2tp.tile([D, 128], BF16, tag="x2t_sb")
                nc.scalar.copy(x2t[:], x2t_ps[:])
                g2ps = ps_sm.tile([128, NI], F32, tag="sm")
                nc.tensor.matmul(g2ps[:], x2t[:], xdt_sb[:, b, :],
                                 start=True, stop=True)
                nc.scalar.copy(g2stack[:, :, b], g2ps[:])

        # ---- outs_2 = W2 : G2 ----
        out2ps = ps_sm.tile([HK, BS], F32, tag="sm")
        for i in range(NI):
            nc.tensor.matmul(out2ps[:], w2sb[:, i, :], g2stack[:, i, :],
                             start=(i == 0), stop=(i == NI - 1))
        nc.scalar.copy(outs_sb[:, 2, :], out2ps[:])

        # ---- transpose [h, b] -> [b, h] and store ----
        for k in range(3):
            trp = ps_sm.tile([128, 128], F32, tag="sm")
            nc.tensor.transpose(trp[:], outs_sb[:, k, :], idf[:])
            nc.scalar.copy(outT_sb[:, k, :], trp[:])
        nc.sync.dma_start(out_d[:], outT_sb[:])

    nc.compile()
    return nc


_II0, _JJ0 = np.triu_indices(M)          # 820 pairs, i <= j


def _host_prep(x, W0, W1, W2):
    """Build per-core input maps. All reshapes/casts in numpy."""
    # layer-0 symmetric weights: W0s[c,h] = W0[i,j,h] + W0[j,i,h] (i<j), diag 1x
    w0sym = W0[_II0, _JJ0, :] + np.where(
        (_II0 != _JJ0)[:, None], W0[_JJ0, _II0, :], 0.0)          # [820, HK]
    w0pad = np.zeros((NCH0 * 128, HK), np.float32)
    w0pad[:NP0] = w0sym
    w0p = np.ascontiguousarray(w0pad.reshape(NCH0, 128, HK)).astype(NPBF16)
    w1t = np.ascontiguousarray(W1.transpose(1, 0, 2)).astype(NPBF16)
    w2t = np.ascontiguousarray(W2.transpose(1, 0, 2)).astype(NPBF16)
    idb = np.eye(128, dtype=np.float32).astype(NPBF16)
    idf = np.eye(128, dtype=np.float32)

    # padded pair index maps (pad rows point at row 0 but weights are zero;
    # use an explicit zero row instead to keep P small and exact)
    ii = np.zeros(NCH0 * 128, np.int64)
    jj = np.zeros(NCH0 * 128, np.int64)
    ii[:NP0] = _II0
    jj[:NP0] = _JJ0
    pad_mask = np.zeros((NCH0 * 128, 1), np.float32)
    pad_mask[:NP0] = 1.0

    xbf = x.astype(NPBF16)
    in_maps = []
    for c in range(NCORE):
        xs = xbf[c * BS:(c + 1) * BS]                     # [BS, M, D]
        xsT = xs.transpose(1, 0, 2)                       # [M, BS, D]
        xf = xsT.reshape(M, NBLK, F).astype(np.float32)   # [M, NBLK, F]
        x0f = np.ascontiguousarray(
            xf.transpose(1, 0, 2)).astype(NPBF16)         # [NBLK, M, F]
        # gather images for layer-0 pairs: [NBLK, c, F] -> [NBLK, g, e, p, F]
        xsuf = (xf[jj] * pad_mask[:, :, None]).transpose(1, 0, 2)
        rsuf = (xf[ii] * pad_mask[:, :, None]).transpose(1, 0, 2)
        xsuf = np.ascontiguousarray(
            xsuf.reshape(NBLK, NGRP0, GRP, 128, F)).astype(NPBF16)
        rsuf = np.ascontiguousarray(
            rsuf.reshape(NBLK, NGRP0, GRP, 128, F)).astype(NPBF16)
        xdt = np.ascontiguousarray(xs.transpose(2, 0, 1))  # [D, BS, M]
        in_maps.append({
            "xsuf": xsuf, "rsuf": rsuf, "x0f": x0f, "xdt": xdt,
            "w0": w0p, "w1": w1t, "w2": w2t,
            "idb": idb, "idf": idf,
        })
    return in_maps


def kernel(x, W0, W1, W2):
    global _nc_cache, _last_results
    x = np.asarray(x, dtype=np.float32)
    W0 = np.asarray(W0, dtype=np.float32)
    W1 = np.asarray(W1, dtype=np.float32)
    W2 = np.asarray(W2, dtype=np.float32)

    if _nc_cache is None:
        _nc_cache = _build()
    nc = _nc_cache

    in_maps = _host_prep(x, W0, W1, W2)
    res = run_bass_kernel_spmd(nc, in_maps, list(range(NCORE)),
                               trace=_PROFILE, **_TRACE_KW)
    _last_results = res
    out = np.concatenate(
        [np.asarray(res.results[c]["out"]) for c in range(NCORE)], axis=0)
    return out.astype(np.float32)
